# revision 65
# baseline (speedup 1.0000x reference)
import numpy as np
import sys
sys.path.insert(0, '/opt/trn_rl_repo')
import jax
try:
    jax.config.update("jax_compilation_cache_dir", "/tmp/jaxcache")
    jax.config.update("jax_persistent_cache_min_compile_time_secs", 0.0)
    jax.config.update("jax_persistent_cache_min_entry_size_bytes", 0)
except Exception:
    pass
import concourse.bacc as bacc
import concourse.mybir as mybir
from concourse.tile import TileContext
from concourse.bass import ds
from concourse.bass_utils import run_bass_kernel_spmd
import concourse.tile_utils as tile_utils
tile_utils.max_sbuf_usage = 207 * 1024

import ml_dtypes
BF = ml_dtypes.bfloat16

TH1 = 2.3599835635698114
TH2 = 7.985043705972782
TH3 = 3.849629060468402
BETA = 0.44154740154430405
EPS = 1e-5
NSTEP = 10
NCORES = 8
B = 512            # batch per core
F32 = mybir.dt.float32
BF16 = mybir.dt.bfloat16

# Even conv-rows first inside each 8-row block so 2x2 pool-y is a single
# quadrant-aligned max(p[0:64], p[64:128]).
PERM8 = (0, 2, 4, 6, 1, 3, 5, 7)

_cache = {}
LAST_RES = None
LAST_NS = -1


def _build_program():
    nc = bacc.Bacc("TRN2", target_bir_lowering=False, debug=False, num_devices=NCORES)

    x26_d = nc.dram_tensor("x26", [26, 26 * B], F32, kind="ExternalInput")
    # aux32 cols: 0:3 bn1 (m,s,b), 3:6 bn2, 6 b_fc (rows 0:10),
    # 7:23 bw1c (rows 3*dx..3*dx+3 = [dyy, co] block for tap dx)
    aux32_d = nc.dram_tensor("aux32", [128, 23], F32, kind="ExternalInput")
    # auxbf cols: 0:96 = bw2c (3 x [48,32], rows 0:48) with wfc4 tucked at rows 64:96
    # of cols 0:50; cols 96:146 = wfc0123
    auxbf_d = nc.dram_tensor("auxbf", [128, 146], BF16, kind="ExternalInput")
    # FC pre-activation pfc (integers in [-800,800]) packed as v = pfc+800 in [0,1600]:
    # cols 0:5120 = low byte of v (col t*B+b), cols 5120:7680 = hi nibbles packed
    # (hi of t<5)*16 + (hi of t>=5), hi = v>>8 in [0,6]. Host replays f32 LIF3 exactly.
    out_d = nc.dram_tensor("out", [10, 15 * B], mybir.dt.uint8, kind="ExternalOutput")

    GT, MUL, ADD, SUB, MAX = (mybir.AluOpType.is_gt, mybir.AluOpType.mult,
                              mybir.AluOpType.add, mybir.AluOpType.subtract,
                              mybir.AluOpType.max)
    F = 12 * B         # LIF1 free width: X' 0..11 (X'=12 never read by conv2)

    with TileContext(nc) as tc:
        with (
            tc.tile_pool(name="state", bufs=1) as st,
            tc.tile_pool(name="xt", bufs=1) as xpool,
            tc.tile_pool(name="tr", bufs=1) as tr,
            tc.tile_pool(name="tq", bufs=1) as tq,
            tc.tile_pool(name="outp", bufs=1) as op,
            tc.tile_pool(name="ps", bufs=2, space="PSUM") as pp,
        ):
            cur1a = st.tile([128, F], F32)      # Y' 0..7 x16ch
            cur1b = st.tile([64, F], F32)       # Y' 8..11
            mem1a = st.tile([128, F], F32)
            mem1b = st.tile([64, F], F32)
            spk1a = st.tile([128, F], BF16)
            spk1b = st.tile([64, F], BF16)
            m2ab = st.tile([128, 5 * B], F32)   # Y2 0,1 | 2,3
            m2c = st.tile([32, 5 * B], F32)     # Y2 4

            aux32 = st.tile([128, 23], F32)
            nc.sync.dma_start(aux32[:], aux32_d[:, 0:23])

            # conv1 stationaries built on device from the 144-value bw1c
            wc1t = st.tile([26, 9 * 128], F32)
            nc.vector.memset(wc1t[:], 0.0)
            for bi, y0 in enumerate((0, 8, 16)):
                for dx in range(3):
                    cb = (bi * 3 + dx) * 128
                    for p_, j in enumerate(PERM8):
                        nc.sync.dma_start(
                            wc1t[y0 + j:y0 + j + 3, cb + p_ * 16:cb + (p_ + 1) * 16],
                            aux32_d[3 * dx:3 * (dx + 1), 7:23])

            # conv2 Toeplitz blocks built on device from auxbf cols 0:96
            w03, w47, w89 = [], [], []
            for dx in range(3):
                ws = slice(dx * 32, (dx + 1) * 32)
                t1 = st.tile([96, 128], BF16, tag=f"w03_{dx}")
                nc.vector.memset(t1[:], 0.0)
                for pos, yr in enumerate((0, 2, 1, 3)):
                    nc.sync.dma_start(t1[yr * 16:yr * 16 + 48, pos * 32:(pos + 1) * 32],
                                      auxbf_d[0:48, ws])
                w03.append(t1)
                # rows for Y 4..7 live at partitions 64..127, Y 8..9 at 0..31
                t2 = st.tile([128, 128], BF16, tag=f"w47_{dx}")
                nc.vector.memset(t2[:], 0.0)
                for pos, yr in enumerate((0, 2, 1, 3)):
                    r0 = yr * 16          # block row where this 48-row strip starts
                    cs = slice(pos * 32, (pos + 1) * 32)
                    lo = min(48, 64 - r0)  # rows landing in block rows <64 -> +64
                    if lo > 0:
                        nc.sync.dma_start(t2[64 + r0:64 + r0 + lo, cs], auxbf_d[0:lo, ws])
                    if lo < 48:
                        nc.sync.dma_start(t2[r0 + lo - 64:r0 + 48 - 64, cs], auxbf_d[lo:48, ws])
                w47.append(t2)
                t3 = st.tile([64, 64], BF16, tag=f"w89_{dx}")
                nc.vector.memset(t3[:], 0.0)
                for pos, yr in enumerate((0, 1)):
                    nc.sync.dma_start(t3[yr * 16:yr * 16 + 48, pos * 32:(pos + 1) * 32],
                                      auxbf_d[0:48, ws])
                w89.append(t3)
            wfc0123 = st.tile([128, 50], BF16)
            nc.sync.dma_start(wfc0123[:], auxbf_d[:, 96:146])
            wfc4 = st.tile([32, 50], BF16)
            nc.sync.dma_start(wfc4[:], auxbf_d[64:96, 0:50])
            nc.vector.memset(mem1a[:], 0.0)
            nc.vector.memset(mem1b[:], 0.0)
            nc.vector.memset(m2ab[:], 0.0)
            nc.vector.memset(m2c[:], 0.0)

            # ---- conv1 + 2x2 pool, on-device; three x-thirds to bound SBUF
            for h in range(3):
                xoff = 8 * h                    # x_in of tile column 0
                x26t = xpool.tile([26, 10 * B], F32, tag="x26")
                nc.sync.dma_start(x26t[:], x26_d[:, xoff * B:(xoff + 10) * B])
                for xp in range(4 * h, 4 * (h + 1)):
                    px0 = tr.tile([128, B], F32, tag="cpx0")
                    px1 = tr.tile([128, B], F32, tag="cpx1")
                    px2 = tr.tile([128, B], F32, tag="cpx2")
                    px = (px0, px1, px2)
                    for xo in range(2):
                        x = 2 * xp + xo
                        for bi in range(3):
                            p = pp.tile([128, B], F32, tag=("p03", "p47", "p89")[bi])
                            for dx in range(3):
                                ci = (x + dx - xoff) * B
                                nc.tensor.matmul(
                                    p[:],
                                    wc1t[:, (bi * 3 + dx) * 128:(bi * 3 + dx + 1) * 128],
                                    x26t[:, ci:ci + B],
                                    start=(dx == 0), stop=(dx == 2))
                            if xo == 0:
                                nc.scalar.copy(px[bi][:], p[:])
                            else:
                                nc.vector.tensor_tensor(px[bi][:], px[bi][:], p[:], op=MAX)
                    xs = slice(xp * B, (xp + 1) * B)
                    odc = tr.tile([64, B], F32, tag="odc")
                    for bi, dest in ((0, cur1a[0:64, xs]),
                                     (1, cur1a[64:128, xs]),
                                     (2, cur1b[0:64, xs])):
                        nc.vector.tensor_copy(odc[:], px[bi][64:128, :])
                        nc.vector.tensor_tensor(dest, px[bi][0:64, :], odc[:], op=MAX)

            # BN1 (pool-before-BN is exact: s1 >= 0)
            nc.vector.tensor_scalar(cur1a[:], cur1a[:], aux32[:, 0:1], aux32[:, 1:2], op0=SUB, op1=MUL)
            nc.vector.tensor_scalar(cur1a[:], cur1a[:], aux32[:, 2:3], None, op0=ADD)
            nc.vector.tensor_scalar(cur1b[:], cur1b[:], aux32[0:64, 0:1], aux32[0:64, 1:2], op0=SUB, op1=MUL)
            nc.vector.tensor_scalar(cur1b[:], cur1b[:], aux32[0:64, 2:3], None, op0=ADD)

            pstore = st.tile([10, NSTEP * B], mybir.dt.float16)

            NCH = 4
            CW = F // NCH
            with tc.For_i(0, NSTEP * B, B) as tv:
                # ---- LIF1
                for (mem, cur, spk, P) in ((mem1a, cur1a, spk1a, 128),
                                           (mem1b, cur1b, spk1b, 64)):
                    for hh in range(NCH):
                        c = slice(hh * CW, (hh + 1) * CW)
                        rs = tq.tile([128, CW], F32, tag="rs")
                        nc.vector.tensor_scalar(rs[:P, :], mem[:, c], TH1, TH1, op0=GT, op1=MUL)
                        nc.vector.tensor_scalar(mem[:, c], mem[:, c], BETA, None, op0=MUL)
                        nc.vector.tensor_tensor(mem[:, c], mem[:, c], cur[:, c], op=ADD)
                        nc.vector.tensor_tensor(mem[:, c], mem[:, c], rs[:P, :], op=SUB)
                        nc.vector.tensor_scalar(spk[:, c], mem[:, c], TH1, None, op0=GT)

                # ---- conv2 + pool + BN2 + LIF2 + FC
                pfc = pp.tile([10, B], F32, tag="pfc")
                for xp in range(5):
                    px03 = tr.tile([128, B], F32, tag="cpx0")
                    px47 = tr.tile([128, B], F32, tag="cpx1")
                    px89 = tr.tile([128, B], F32, tag="cpx2")
                    for xo in range(2):
                        x = 2 * xp + xo
                        p03 = pp.tile([128, B], F32, tag="p03")
                        p47 = pp.tile([128, B], F32, tag="p47")
                        p89 = pp.tile([128, B], F32, tag="p89")
                        for dx in range(3):
                            Xd = slice((x + dx) * B, (x + dx + 1) * B)
                            nc.tensor.matmul(p03[:], w03[dx][:], spk1a[0:96, Xd],
                                             start=(dx == 0), stop=(dx == 2))
                            nc.tensor.matmul(p47[:], w47[dx][64:128, :], spk1a[64:128, Xd],
                                             start=(dx == 0), stop=False)
                            nc.tensor.matmul(p47[:], w47[dx][0:32, :], spk1b[0:32, Xd],
                                             start=False, stop=(dx == 2))
                            nc.tensor.matmul(p89[0:64, :], w89[dx][:], spk1b[0:64, Xd],
                                             start=(dx == 0), stop=(dx == 2))
                        if xo == 0:
                            nc.scalar.copy(px03[:], p03[:])
                            nc.scalar.copy(px47[:], p47[:])
                            nc.scalar.copy(px89[0:64, :], p89[0:64, :])
                        else:
                            nc.vector.tensor_tensor(px03[:], px03[:], p03[:], op=MAX)
                            nc.vector.tensor_tensor(px47[:], px47[:], p47[:], op=MAX)
                            nc.vector.tensor_tensor(px89[0:64, :], px89[0:64, :], p89[0:64, :], op=MAX)
                    xsg = slice(xp * B, (xp + 1) * B)
                    first = (xp == 0)
                    plt = tr.tile([128, B], F32, tag="pl")
                    rs2t = tr.tile([128, B], F32, tag="rs2")
                    spk2t = tr.tile([128, B], BF16, tag="spk2")
                    od2 = tr.tile([64, B], F32, tag="od2")
                    for gi, (pxt, m2g, wfct, sl, gp) in enumerate((
                            (px03, m2ab[0:64, xsg], wfc0123, slice(0, 64), 64),
                            (px47, m2ab[64:128, xsg], wfc0123, slice(64, 128), 64),
                            (px89, m2c[0:32, xsg], wfc4, slice(0, 32), 32))):
                        nc.vector.tensor_copy(od2[0:gp, :], pxt[gp:2 * gp, :])
                        nc.vector.tensor_tensor(plt[sl, :], pxt[0:gp, :], od2[0:gp, :], op=MAX)
                        # BN2: (k - m) * s + b
                        nc.vector.tensor_scalar(plt[sl, :], plt[sl, :],
                                                aux32[sl, 3:4], aux32[sl, 4:5],
                                                op0=SUB, op1=MUL)
                        nc.vector.tensor_scalar(plt[sl, :], plt[sl, :],
                                                aux32[sl, 5:6], None, op0=ADD)
                        # LIF2
                        nc.vector.tensor_scalar(rs2t[sl, :], m2g, TH2, TH2, op0=GT, op1=MUL)
                        nc.vector.tensor_scalar(m2g, m2g, BETA, None, op0=MUL)
                        nc.vector.tensor_tensor(m2g, m2g, plt[sl, :], op=ADD)
                        nc.vector.tensor_tensor(m2g, m2g, rs2t[sl, :], op=SUB)
                        nc.vector.tensor_scalar(spk2t[sl, :], m2g, TH2, None, op0=GT)
                        nc.tensor.matmul(pfc[:], wfct[sl, 10 * xp:10 * xp + 10], spk2t[sl, :],
                                         start=(first and gi == 0),
                                         stop=(xp == 4 and gi == 2))

                # ---- record FC pre-activation (integer-valued, fp16-exact)
                nc.vector.tensor_copy(pstore[:, ds(tv, B)], pfc[:])

            # pack v = pstore+800 in [0,1600] into u8 low bytes + paired hi nibbles,
            # all exact int arithmetic (thresholds at .5, power-of-2 scales)
            HB = NSTEP * B // 2
            CK = B                         # chunk width (divides HB)
            for k in range(HB // CK):
                ca = slice(k * CK, (k + 1) * CK)
                cb = slice(HB + k * CK, HB + (k + 1) * CK)
                hsum = []
                for half, cs in ((0, ca), (1, cb)):
                    v = op.tile([10, CK], F32, tag="pkv")
                    nc.vector.tensor_scalar(v[:], pstore[:, cs], 800.0, None, op0=ADD)
                    bs = op.tile([10, CK], F32, tag=f"pkb{half}")
                    tm = op.tile([10, CK], F32, tag="pkt")
                    nc.vector.tensor_scalar(bs[:], v[:], 1023.5, 1024.0, op0=GT, op1=MUL)
                    nc.vector.tensor_tensor(v[:], v[:], bs[:], op=SUB)
                    for thr, sc in ((511.5, 512.0), (255.5, 256.0)):
                        nc.vector.tensor_scalar(tm[:], v[:], thr, sc, op0=GT, op1=MUL)
                        nc.vector.tensor_tensor(v[:], v[:], tm[:], op=SUB)
                        nc.vector.tensor_tensor(bs[:], bs[:], tm[:], op=ADD)
                    lou8 = op.tile([10, CK], mybir.dt.uint8, tag="lou8")
                    nc.vector.tensor_copy(lou8[:], v[:])           # low byte, exact
                    nc.sync.dma_start(out_d[:, cs], lou8[:])
                    hsum.append(bs)                                # 256 * hi
                hp = op.tile([10, CK], F32, tag="pkt")
                nc.vector.tensor_scalar(hp[:], hsum[0][:], 16.0 / 256.0, None, op0=MUL)
                tm2 = op.tile([10, CK], F32, tag="pkv")
                nc.vector.tensor_scalar(tm2[:], hsum[1][:], 1.0 / 256.0, None, op0=MUL)
                nc.vector.tensor_tensor(hp[:], hp[:], tm2[:], op=ADD)
                hiu8 = op.tile([10, CK], mybir.dt.uint8, tag="hiu8")
                nc.vector.tensor_copy(hiu8[:], hp[:])
                nc.sync.dma_start(out_d[:, 10 * B + k * CK:10 * B + (k + 1) * CK], hiu8[:])

    nc.compile()
    return nc


def kernel(inpt, w1, w2, w_fc, b_fc, bn1_g, bn1_b, bn1_m, bn1_v,
           bn2_g, bn2_b, bn2_m, bn2_v):
    inpt = np.asarray(inpt, np.float32)
    w1 = np.asarray(w1, np.float32); w2 = np.asarray(w2, np.float32)
    w_fc = np.asarray(w_fc, np.float32); b_fc = np.asarray(b_fc, np.float32)
    bn1_g = np.asarray(bn1_g, np.float32); bn1_b = np.asarray(bn1_b, np.float32)
    bn1_m = np.asarray(bn1_m, np.float32); bn1_v = np.asarray(bn1_v, np.float32)
    bn2_g = np.asarray(bn2_g, np.float32); bn2_b = np.asarray(bn2_b, np.float32)
    bn2_m = np.asarray(bn2_m, np.float32); bn2_v = np.asarray(bn2_v, np.float32)

    bw1 = np.sign(w1).astype(np.float32)
    bw2 = np.sign(w2).astype(np.float32)
    bwfc = np.sign(w_fc).astype(np.float32)
    s1 = (bn1_g * (np.float32(1.0) / np.sqrt(bn1_v + EPS, dtype=np.float32))).astype(np.float32)
    s2 = (bn2_g * (np.float32(1.0) / np.sqrt(bn2_v + EPS, dtype=np.float32))).astype(np.float32)

    # compact weight sources; Toeplitz blocks are assembled on device via DMA
    bw1c = np.ascontiguousarray(bw1[:, 0].transpose(2, 1, 0))          # [dx, dyy, co]
    bw2c = np.ascontiguousarray(bw2.transpose(3, 2, 1, 0)).reshape(3, 48, 32)

    aux32 = np.zeros((128, 23), np.float32)
    aux32[:, 0] = np.tile(bn1_m, 8); aux32[:, 1] = np.tile(s1, 8); aux32[:, 2] = np.tile(bn1_b, 8)
    aux32[:, 3] = np.tile(bn2_m, 4); aux32[:, 4] = np.tile(s2, 4); aux32[:, 5] = np.tile(bn2_b, 4)
    aux32[0:10, 6] = b_fc
    aux32[0:9, 7:23] = bw1c.reshape(9, 16)

    wfc_r = bwfc.reshape(10, 32, 5, 5)
    def fcblock(yps):
        W = np.zeros((len(yps) * 32, 50), np.float32)
        for i, yp in enumerate(yps):
            W[i * 32:(i + 1) * 32] = wfc_r[:, :, yp, :].transpose(1, 2, 0).reshape(32, 50)
        return W.astype(BF)
    auxbf = np.zeros((128, 146), BF)
    for dx in range(3):
        auxbf[0:48, dx * 32:(dx + 1) * 32] = bw2c[dx]
    auxbf[:, 96:146] = np.vstack([fcblock([0, 1]), fcblock([2, 3])])
    auxbf[64:96, 0:50] = fcblock([4])

    if 'nc' not in _cache:
        nc = _build_program()
        # The BIR is final after compile(); memoize its serialization so the
        # per-call jax lowering doesn't re-serialize the identical module.
        _bir_bytes = nc.to_json_bytes()
        nc.to_json_bytes = lambda _b=_bir_bytes: _b
        _cache['nc'] = nc
    nc = _cache['nc']

    XT = np.ascontiguousarray(inpt[:, 0, 0:26, 0:26].transpose(1, 2, 0))  # [26,26,Bfull]
    in_maps = []
    for c in range(NCORES):
        xc = np.ascontiguousarray(XT[:, :, c * B:(c + 1) * B]).reshape(26, 26 * B)
        in_maps.append({
            "x26": xc, "aux32": aux32, "auxbf": auxbf,
        })

    import time as _time
    _t0 = _time.perf_counter()
    res = run_bass_kernel_spmd(nc, in_maps, list(range(NCORES)))
    _t1 = _time.perf_counter()
    global LAST_RES, LAST_NS
    LAST_RES = res
    LAST_NS = (_t1 - _t0) * 1e9
    arr = np.stack([np.asarray(r["out"]) for r in res.results]).astype(np.int32)  # [8,10,7680] u8
    # unpack v = pfc+800: low bytes at cols 0:5120 (t*B+b), hi nibbles packed pairwise
    lo = arr[:, :, 0:NSTEP * B].reshape(NCORES, 10, NSTEP, B)
    h2 = arr[:, :, NSTEP * B:].reshape(NCORES, 10, NSTEP // 2, B)
    hi = np.concatenate([h2 >> 4, h2 & 15], axis=2)
    pfc = (lo + 256 * hi - 800).astype(np.float32)                 # [core, cls, t, b]
    cur3 = np.ascontiguousarray(pfc.transpose(2, 0, 3, 1).reshape(NSTEP, NCORES * B, 10))
    mem3 = np.zeros((NCORES * B, 10), np.float32)
    spk = np.empty((NSTEP, NCORES * B, 10), np.float32)
    mem = np.empty_like(spk)
    for t in range(NSTEP):
        c3 = (cur3[t] + b_fc[None, :]).astype(np.float32)
        r3 = (mem3 > TH3).astype(np.float32)
        mem3 = (BETA * mem3 + c3 - r3 * TH3).astype(np.float32)
        spk[t] = (mem3 > TH3).astype(np.float32)
        mem[t] = mem3
    return spk, mem


if __name__ == "__main__":
    pass
